# revision 1
# baseline (speedup 1.0000x reference)
"""Trainium2 Bass kernel for the quantized ResNet bottleneck block.

Data-parallel over batch: 64 images -> 8 cores x 8 images.

Per-core pipeline (all layouts channel-major [C_part, pix] except layer 3):
  conv1: fp32 matmuls (K=512 in 4 k-tiles), rhs = xb3 (= x + bn3_beta, see below)
  bn1+relu: ScalarE activation (per-partition scale/bias)
  bfp quant: reduce(apply_transpose) -> block max; exponent bit-math -> delta;
             delta replicated across the 32 channels of each block via a
             broadcast-read v.transpose; one custom fused DVE op does
             round/clip/rescale via the scaled-magic-constant trick.
  conv2: 3x3 via 9 shifted-window matmuls on a zero-padded buffer, weights
         split hi/lo in float32r (2 matmuls/tap, full fp32 precision at
         1 cyc/row).
  conv3: pixel-major orientation (lhsT = activations), N=512 out-channels,
         fp32r hi/lo weights; residual add = one tensor_tensor add against a
         pre-transposed xb3_T; final bfp quant needs no transposes.

bn3 trick: the kernel consumes xb3 = x + bn3_beta instead of x, so the
residual add needs no extra bias op; conv1's bias is corrected by
-inv1 * (w1q @ bn3_beta) on the host.
Output is produced pixel-major [pix, C] and transposed back on the host.
"""
import numpy as np
import ml_dtypes
from contextlib import ExitStack

import concourse.bass as bass
import concourse.bacc as bacc
import concourse.tile as tile
from concourse import mybir
from concourse.bass_utils import run_bass_kernel_spmd

F32 = mybir.dt.float32
F32R = mybir.dt.float32r
AL = mybir.AluOpType
AFT = mybir.ActivationFunctionType

# ---------------- custom DVE op: fused bfp round/clip/rescale ---------------
# out = min(max(in0 + in1*M, in1*M), in1*(M+127)) - in1*M
# with in1 = delta (power of two).  Adding M*delta rounds in0 to the delta
# grid (round-half-even); the clips implement relu and the 127 cap; the
# subtract is exact (Sterbenz).  M = 1.5 * 2^23.
import concourse.dve_ops as dve_ops
from concourse.dve_spec import Spec, Src0, Src1, C0, C1, minn, maxx

MAGIC = 12582912.0

def _bfp_ref(in0, in1, s0, s1, imm2):
    lo = in1 * s0
    return (np.minimum(np.maximum(in0 + lo, lo), in1 * s1) - lo).astype(np.float32)

BFP_QUANT_ANT = dve_ops.DveOp(
    "BFP_QUANT_ANT",
    Spec(
        body=minn(maxx(Src0 + Src1 * C0, Src1 * C0), Src1 * C1) - Src1 * C0,
        reference=_bfp_ref,
    ),
    subdim=False,
    uops_sha={"v3": "09229989be91bde3", "v4": "701a1ee7014b78c5"},
)

def _register_bfp_op():
    if "BFP_QUANT_ANT" not in dve_ops._SUB_OPCODE_FOR_NAME:
        dve_ops.OPS.append(BFP_QUANT_ANT)
        dve_ops.CUSTOM_DVE_SPECS["BFP_QUANT_ANT"] = BFP_QUANT_ANT.spec
        dve_ops._SUB_OPCODE_FOR_NAME["BFP_QUANT_ANT"] = (
            dve_ops._CUSTOM_DVE_ROW_BASE + len(dve_ops.OPS) - 1)

_register_bfp_op()

# ---------------- geometry (hardcoded for this problem) ---------------------
N_IMG = 8          # images per core
CIN = 512
WID = 128
H = W = 28
HW = H * W         # 784
PIX = N_IMG * HW   # 6272
PADH = PADW = 30
PADHW = PADH * PADW  # 900
NT392 = 392        # conv N-tile (14 rows)
GRP = 1568         # quant group = 2 images


def _emit_delta_math(nc, rmax):
    """In-place on rmax AP [128, nb] f32: delta = 2^(floor(log2(max(rmax,1e-24)))-6)."""
    nc.vector.tensor_scalar_max(rmax, rmax, 1e-24)
    nc.vector.tensor_scalar(rmax.bitcast(mybir.dt.int32), rmax.bitcast(mybir.dt.int32),
                            0x7F800000, None, op0=AL.bitwise_and)
    nc.vector.tensor_scalar_mul(rmax, rmax, 0.015625)


def build_nc():
    nc = bacc.Bacc()

    xb3 = nc.declare_dram_parameter("xb3", [N_IMG, CIN, HW], F32, False)
    xTh = nc.declare_dram_parameter("xTh", [PIX, CIN], mybir.dt.bfloat16, False)
    xTl = nc.declare_dram_parameter("xTl", [PIX, CIN], mybir.dt.bfloat16, False)
    ident = nc.declare_dram_parameter("ident", [128, 128], mybir.dt.bfloat16, False)
    w1T = nc.declare_dram_parameter("w1T", [CIN, WID], F32, False)
    w2T = nc.declare_dram_parameter("w2T", [9, WID, WID], F32, False)
    w3T = nc.declare_dram_parameter("w3T", [WID, CIN], F32, False)
    inv1 = nc.declare_dram_parameter("inv1", [WID, 1], F32, False)
    bet1 = nc.declare_dram_parameter("bet1", [WID, 1], F32, False)
    inv2 = nc.declare_dram_parameter("inv2", [WID, 1], F32, False)
    bet2 = nc.declare_dram_parameter("bet2", [WID, 1], F32, False)
    outT = nc.declare_dram_parameter("outT", [PIX, CIN], F32, True)

    with tile.TileContext(nc) as tc, ExitStack() as ctx:
        wp = ctx.enter_context(tc.tile_pool(name="wp", bufs=1))
        big = ctx.enter_context(tc.tile_pool(name="big", bufs=1))
        stage = ctx.enter_context(tc.tile_pool(name="stage", bufs=3))
        ygp = ctx.enter_context(tc.tile_pool(name="ygp", bufs=4))
        xs = ctx.enter_context(tc.tile_pool(name="xs", bufs=16))
        xt3 = ctx.enter_context(tc.tile_pool(name="xt3", bufs=4))
        dsm = ctx.enter_context(tc.tile_pool(name="dsm", bufs=4))
        pp = ctx.enter_context(tc.tile_pool(name="pp", bufs=3, space="PSUM"))
        p3p = ctx.enter_context(tc.tile_pool(name="p3p", bufs=5, space="PSUM"))

        # ---- params in ----
        w1sb = wp.tile([128, 4, WID], F32)
        nc.sync.dma_start(w1sb[:], w1T[:].rearrange("(k c) o -> c k o", c=128))
        w2sb = wp.tile([128, 9, WID], F32)
        nc.sync.dma_start(w2sb[:], w2T[:].rearrange("t c o -> c t o"))
        w3sb = wp.tile([128, CIN], F32)
        nc.sync.dma_start(w3sb[:], w3T[:])
        identsb = wp.tile([128, 128], mybir.dt.bfloat16)
        nc.sync.dma_start(identsb[:], ident[:])
        bn1s = wp.tile([128, 1], F32); nc.sync.dma_start(bn1s[:], inv1[:])
        bn1b = wp.tile([128, 1], F32); nc.sync.dma_start(bn1b[:], bet1[:])
        bn2s = wp.tile([128, 1], F32); nc.sync.dma_start(bn2s[:], inv2[:])
        bn2b = wp.tile([128, 1], F32); nc.sync.dma_start(bn2b[:], bet2[:])

        # weight hi/lo splits (bf16), emitted lazily after layer-1 starts
        w2hi = wp.tile([128, 9, WID], mybir.dt.bfloat16)
        w2lo = wp.tile([128, 9, WID], mybir.dt.bfloat16)
        tmp2 = wp.tile([128, 9, WID], F32)
        w3hi = wp.tile([128, CIN], mybir.dt.bfloat16)
        w3lo = wp.tile([128, CIN], mybir.dt.bfloat16)
        tmp3 = wp.tile([128, CIN], F32)

        def emit_wsplits():
            nc.vector.tensor_copy(w2hi[:], w2sb[:])
            nc.vector.tensor_sub(tmp2[:], w2sb[:], w2hi[:])
            nc.vector.tensor_copy(w2lo[:], tmp2[:])
            nc.vector.tensor_copy(w3hi[:], w3sb[:])
            nc.vector.tensor_sub(tmp3[:], w3sb[:], w3hi[:])
            nc.vector.tensor_copy(w3lo[:], tmp3[:])

        # ---- activations / residual state ----
        a1pad = big.tile([128, N_IMG, PADH, PADW], mybir.dt.bfloat16)
        nc.gpsimd.memset(a1pad[:].rearrange("p n h w -> p (n h w)").bitcast(mybir.dt.int32), 0)
        a2 = big.tile([128, PIX], mybir.dt.bfloat16)

        # ================= emit functions =================
        taps = [(dy, dx) for dy in range(3) for dx in range(3)]

        def emit_l1(g):
            ygrp = ygp.tile([128, GRP], F32, tag="ygrp")
            for si in range(4):
                n = 2 * g + si // 2
                q0 = NT392 * (si % 2)
                xk = []
                for k in range(4):
                    xt = xs.tile([128, NT392], F32, tag="xk")
                    eng = nc.sync if (k % 2 == 0) else nc.scalar
                    eng.dma_start(xt[:], xb3[n, 128*k:128*(k+1), q0:q0+NT392])
                    xk.append(xt)
                pst = pp.tile([128, CIN], F32, tag="cp")
                ps = pst[:, :NT392]
                for k in range(4):
                    nc.tensor.matmul(ps[:], w1sb[:, k, :], xk[k][:, :],
                                     start=(k == 0), stop=(k == 3))
                nc.scalar.activation(ygrp[:, si*NT392:(si+1)*NT392], ps[:], AFT.Relu,
                                     bias=bn1b[:], scale=bn1s[:])
            rmax = dsm.tile([128, 49], F32, tag="rmax")
            nc.vector.tensor_reduce(rmax[:], ygrp[:].rearrange("p (b j) -> p b j", b=49, j=32),
                                    axis=mybir.AxisListType.X, op=AL.max,
                                    apply_transpose=True)
            _emit_delta_math(nc, rmax[:])
            dcm = dsm.tile([128, GRP], F32, tag="dcm")
            nc.vector.transpose(dcm[:], rmax[:].unsqueeze(2).broadcast_to([128, 49, 32]))
            for im in range(2):
                n = 2 * g + im
                nc.vector._custom_dve(
                    BFP_QUANT_ANT,
                    out=a1pad[:, n, 1:29, 1:29],
                    in0=ygrp[:, im*HW:(im+1)*HW],
                    in1=dcm[:, im*HW:(im+1)*HW],
                    s0=MAGIC, s1=MAGIC + 127.0,
                )

        def emit_l2(g):
            ygrp = ygp.tile([128, GRP], F32, tag="y2grp")
            for si in range(4):
                n = 2 * g + si // 2
                h0 = 14 * (si % 2)
                pst = pp.tile([128, CIN], F32, tag="cp")
                ps = pst[:, :NT392]
                i = 0
                for t, (dy, dx) in enumerate(taps):
                    for wt in (w2hi, w2lo):
                        rhs = a1pad[:, n, h0+dy:h0+dy+14, dx:dx+28]
                        nc.tensor.matmul(ps[:], wt[:, t, :], rhs,
                                         start=(i == 0), stop=(i == 17))
                        i += 1
                nc.scalar.activation(ygrp[:, si*NT392:(si+1)*NT392], ps[:], AFT.Relu,
                                     bias=bn2b[:], scale=bn2s[:])
            rmax = dsm.tile([128, 49], F32, tag="rmax")
            nc.vector.tensor_reduce(rmax[:], ygrp[:].rearrange("p (b j) -> p b j", b=49, j=32),
                                    axis=mybir.AxisListType.X, op=AL.max,
                                    apply_transpose=True)
            _emit_delta_math(nc, rmax[:])
            dcm = dsm.tile([128, GRP], F32, tag="dcm")
            nc.vector.transpose(dcm[:], rmax[:].unsqueeze(2).broadcast_to([128, 49, 32]))
            for im in range(2):
                nc.vector._custom_dve(
                    BFP_QUANT_ANT,
                    out=a2[:, (2*g+im)*HW:(2*g+im+1)*HW],
                    in0=ygrp[:, im*HW:(im+1)*HW],
                    in1=dcm[:, im*HW:(im+1)*HW],
                    s0=MAGIC, s1=MAGIC + 127.0,
                )

        def emit_l3(t0, gn):
            nf = gn * CIN
            xh = xt3.tile([128, 2 * CIN], mybir.dt.bfloat16, tag="xh")
            nc.scalar.dma_start(xh[:, :nf].rearrange("p (j c) -> p j c", j=gn, c=CIN),
                              xTh[128*t0:128*t0 + 128*gn, :].rearrange("(j p) c -> p j c", p=128))
            xl = xt3.tile([128, 2 * CIN], mybir.dt.bfloat16, tag="xl")
            nc.scalar.dma_start(xl[:, :nf].rearrange("p (j c) -> p j c", j=gn, c=CIN),
                              xTl[128*t0:128*t0 + 128*gn, :].rearrange("(j p) c -> p j c", p=128))
            rm3 = dsm.tile([128, 2 * 16], F32, tag="rm3")
            pss = []
            for j in range(gn):
                ps3 = p3p.tile([128, CIN], F32, tag="c3g")
                a2t = a2[:, 128*(t0+j):128*(t0+j+1)]
                nc.tensor.matmul(ps3[:], a2t, w3hi[:], start=True, stop=False)
                nc.tensor.matmul(ps3[:], a2t, w3lo[:], start=False, stop=False)
                nc.tensor.matmul(ps3[:], identsb[:], xh[:, j*CIN:(j+1)*CIN],
                                 start=False, stop=False)
                nc.tensor.matmul(ps3[:], identsb[:], xl[:, j*CIN:(j+1)*CIN],
                                 start=False, stop=True)
                nc.vector.tensor_reduce(rm3[:, j*16:(j+1)*16],
                                        ps3[:].rearrange("p (b k) -> p b k", b=16, k=32),
                                        axis=mybir.AxisListType.X, op=AL.max)
                pss.append(ps3)
            _emit_delta_math(nc, rm3[:, :16*gn])
            o3 = stage.tile([128, 2 * CIN], F32, tag="o3")
            for j in range(gn):
                nc.vector._custom_dve(
                    BFP_QUANT_ANT,
                    out=o3[:, j*CIN:(j+1)*CIN].rearrange("p (b k) -> p b k", b=16, k=32),
                    in0=pss[j][:].rearrange("p (b k) -> p b k", b=16, k=32),
                    in1=rm3[:, j*16:(j+1)*16].unsqueeze(2).broadcast_to([128, 16, 32]),
                    s0=MAGIC, s1=MAGIC + 127.0,
                )
            nc.scalar.dma_start(outT[128*t0:128*t0 + 128*gn, :].rearrange("(j p) c -> p j c", p=128),
                              o3[:, :nf].rearrange("p (j c) -> p j c", j=gn, c=CIN))

        # ================= interleaved schedule =================
        l3g = [(2*i, min(2, 49 - 2*i)) for i in range((49 + 1) // 2)]
        emit_l1(0)
        emit_wsplits()
        emit_l1(1)
        emit_l2(0)
        emit_l1(2)
        for t0, gn in l3g[:6]:      # needs quant2(0) only
            emit_l3(t0, gn)
        emit_l2(1)
        emit_l1(3)
        for t0, gn in l3g[6:12]:    # needs quant2(1)
            emit_l3(t0, gn)
        emit_l2(2)
        for t0, gn in l3g[12:18]:   # needs quant2(2)
            emit_l3(t0, gn)
        emit_l2(3)
        for t0, gn in l3g[18:]:
            emit_l3(t0, gn)

    nc.finalize()
    return nc


# ---------------- host-side parameter prep ---------------------------------
def _w_quant_np(w, blk=32):
    O, I, kh, kw = w.shape
    wb = w.reshape(O, I // blk, blk, kh, kw)
    alpha = np.maximum(np.abs(wb).max(axis=2, keepdims=True) / np.float32(127.0),
                       np.float32(1e-24)).astype(np.float32)
    q = (np.round(wb / alpha) * alpha).astype(np.float32)
    return q.reshape(O, I, kh, kw)


def _bn_fold(g, b, m, v):
    inv = (g / np.sqrt(v + np.float32(1e-5))).astype(np.float32)
    beta = (b - m * inv).astype(np.float32)
    return inv, beta


_NC_CACHE = {}

def kernel(x, w1, w2, w3,
           bn1_g, bn1_b, bn1_m, bn1_v,
           bn2_g, bn2_b, bn2_m, bn2_v,
           bn3_g, bn3_b, bn3_m, bn3_v,
           _want_trace=False):
    x = np.asarray(x, np.float32)
    w1q = _w_quant_np(np.asarray(w1, np.float32))
    w2q = _w_quant_np(np.asarray(w2, np.float32))
    w3q = _w_quant_np(np.asarray(w3, np.float32))
    inv1, bet1 = _bn_fold(*[np.asarray(a, np.float32) for a in (bn1_g, bn1_b, bn1_m, bn1_v)])
    inv2, bet2 = _bn_fold(*[np.asarray(a, np.float32) for a in (bn2_g, bn2_b, bn2_m, bn2_v)])
    inv3, bet3 = _bn_fold(*[np.asarray(a, np.float32) for a in (bn3_g, bn3_b, bn3_m, bn3_v)])

    # bn3 beta folded into the residual input; conv1 bias corrected for it
    xb3 = (x + bet3[None, :, None, None]).astype(np.float32)
    K = (w1q[:, :, 0, 0].astype(np.float64) @ bet3.astype(np.float64))
    bet1c = (bet1.astype(np.float64) - inv1.astype(np.float64) * K).astype(np.float32)

    w1T = np.ascontiguousarray(w1q[:, :, 0, 0].T)                     # [512, 128]
    w2T = np.ascontiguousarray(w2q.transpose(2, 3, 1, 0).reshape(9, WID, WID))
    w3f = (w3q[:, :, 0, 0] * inv3[:, None]).astype(np.float32)
    w3T = np.ascontiguousarray(w3f.T)                                 # [128, 512]

    xb3v = xb3.reshape(64, CIN, HW)
    xTv = np.ascontiguousarray(xb3.reshape(64, CIN, HW).transpose(0, 2, 1))  # [64, 784, 512]

    if "nc" not in _NC_CACHE:
        _NC_CACHE["nc"] = build_nc()
    nc = _NC_CACHE["nc"]

    shared = dict(
        w1T=w1T, w2T=w2T, w3T=w3T,
        ident=np.eye(128, dtype=ml_dtypes.bfloat16),
        inv1=inv1.reshape(WID, 1), bet1=bet1c.reshape(WID, 1),
        inv2=inv2.reshape(WID, 1), bet2=bet2.reshape(WID, 1),
    )
    in_maps = []
    for c in range(8):
        m = dict(shared)
        m["xb3"] = np.ascontiguousarray(xb3v[8*c:8*(c+1)])
        xTc = np.ascontiguousarray(xTv[8*c:8*(c+1)].reshape(PIX, CIN))
        xh_ = xTc.astype(ml_dtypes.bfloat16)
        m["xTh"] = xh_
        m["xTl"] = (xTc - xh_.astype(np.float32)).astype(ml_dtypes.bfloat16)
        in_maps.append(m)

    res = run_bass_kernel_spmd(nc, in_maps, list(range(8)), trace=_want_trace)
    out = np.empty((64, CIN, H, W), np.float32)
    for c in range(8):
        oT = res.results[c]["outT"].reshape(N_IMG, HW, CIN)
        out[8*c:8*(c+1)] = oT.transpose(0, 2, 1).reshape(N_IMG, CIN, H, W)
    if _want_trace:
        return out, res
    return out



# revision 9
# speedup vs baseline: 1.3439x; 1.3439x over previous
"""Trainium2 Bass kernel for the quantized ResNet bottleneck block.

Data-parallel over batch: 64 images -> 8 cores x 8 images.

v2 pipeline (per core, all layouts channel-major [C_part, pix] except l3):
  conv1: bf16 matmuls, w1 hi/lo split x single-bf16 x (8 mm / si-tile).
  bn1+relu: ScalarE activation -> fp16.
  bfp quant: fp16 transposed block-max -> exponent bit-math (int16) -> delta;
             broadcast-transpose back; fused DVE magic-round op -> bf16 a1.
  conv2: 3x3 via 9 shifted-window matmuls, w2 hi/lo bf16 (18 mm / si-tile).
  conv3: pixel-major (lhsT = a2 tile), w3 hi/lo bf16; residual add via one
         bf16 identity matmul (xT bf16); Scalar evacuates PSUM as relu->fp16;
         block-max on fp16; delta in fp32 bit-math; magic-round op emits the
         final quantized VALUES in fp16 (exact: m*delta, m<=127).
Host only casts fp16 -> fp32 and transposes back.

bn3 trick: kernel consumes xb3 = x + bn3_beta; conv1 bias corrected by
-inv1 * (w1q @ bn3_beta) on the host.
"""
import numpy as np
import ml_dtypes
from contextlib import ExitStack

import concourse.bass as bass
import concourse.bacc as bacc
import concourse.tile as tile
from concourse import mybir
from concourse.bass_utils import run_bass_kernel_spmd

F32 = mybir.dt.float32
F16 = mybir.dt.float16
BF16 = mybir.dt.bfloat16
I16 = mybir.dt.int16
I32 = mybir.dt.int32
AL = mybir.AluOpType
AFT = mybir.ActivationFunctionType

# precision toggles (True = exact hi/lo split, False = single bf16)
W1_LO = True
W2_LO = True
W3_LO = True

# ---------------- custom DVE op: fused bfp round/clip/rescale ---------------
# out = min(max(in0 + in1*M, in1*M), in1*(M+127)) - in1*M
# with in1 = delta (power of two).  Adding M*delta rounds in0 to the delta
# grid (round-half-even); the clips implement relu and the 127 cap; the
# subtract is exact (Sterbenz).  M = 1.5 * 2^23 (DVE ALUs compute in fp32).
import concourse.dve_ops as dve_ops
from concourse.dve_spec import Spec, Src0, Src1, C0, C1, minn, maxx

MAGIC = 12582912.0

def _bfp_ref(in0, in1, s0, s1, imm2):
    lo = in1 * s0
    return (np.minimum(np.maximum(in0 + lo, lo), in1 * s1) - lo).astype(np.float32)

BFP_QUANT_ANT = dve_ops.DveOp(
    "BFP_QUANT_ANT",
    Spec(
        body=minn(maxx(Src0 + Src1 * C0, Src1 * C0), Src1 * C1) - Src1 * C0,
        reference=_bfp_ref,
    ),
    subdim=False,
    uops_sha={"v3": "09229989be91bde3", "v4": "701a1ee7014b78c5"},
)

def _register_bfp_op():
    if "BFP_QUANT_ANT" not in dve_ops._SUB_OPCODE_FOR_NAME:
        dve_ops.OPS.append(BFP_QUANT_ANT)
        dve_ops.CUSTOM_DVE_SPECS["BFP_QUANT_ANT"] = BFP_QUANT_ANT.spec
        dve_ops._SUB_OPCODE_FOR_NAME["BFP_QUANT_ANT"] = (
            dve_ops._CUSTOM_DVE_ROW_BASE + len(dve_ops.OPS) - 1)

_register_bfp_op()

# ---------------- geometry (hardcoded for this problem) ---------------------
N_IMG = 8          # images per core
CIN = 512
WID = 128
H = W = 28
HW = H * W         # 784
PIX = N_IMG * HW   # 6272
PADH = PADW = 30
NT392 = 392        # conv N-tile (14 rows)
GRP = 1568         # quant group = 2 images
RMIN = 0.02        # block-max clamp: rmax<RMIN gets delta=2^-12 (error <=2^-13)


def build_nc():
    nc = bacc.Bacc()

    xh = nc.declare_dram_parameter("xh", [N_IMG, CIN, HW], BF16, False)
    xT = nc.declare_dram_parameter("xT", [PIX, CIN], BF16, False)
    ident = nc.declare_dram_parameter("ident", [128, 128], BF16, False)
    w1h = nc.declare_dram_parameter("w1h", [CIN, WID], BF16, False)
    w1l = nc.declare_dram_parameter("w1l", [CIN, WID], BF16, False)
    w2h = nc.declare_dram_parameter("w2h", [9, WID, WID], BF16, False)
    w2l = nc.declare_dram_parameter("w2l", [9, WID, WID], BF16, False)
    w3h = nc.declare_dram_parameter("w3h", [WID, CIN], BF16, False)
    w3l = nc.declare_dram_parameter("w3l", [WID, CIN], BF16, False)
    inv1 = nc.declare_dram_parameter("inv1", [WID, 1], F32, False)
    bet1 = nc.declare_dram_parameter("bet1", [WID, 1], F32, False)
    inv2 = nc.declare_dram_parameter("inv2", [WID, 1], F32, False)
    bet2 = nc.declare_dram_parameter("bet2", [WID, 1], F32, False)
    outV = nc.declare_dram_parameter("outV", [PIX, CIN], F16, True)

    with tile.TileContext(nc) as tc, ExitStack() as ctx:
        wp = ctx.enter_context(tc.tile_pool(name="wp", bufs=1))
        big = ctx.enter_context(tc.tile_pool(name="big", bufs=1))
        ygp = ctx.enter_context(tc.tile_pool(name="ygp", bufs=4))
        xs = ctx.enter_context(tc.tile_pool(name="xs", bufs=16))
        xt3 = ctx.enter_context(tc.tile_pool(name="xt3", bufs=3))
        y3p = ctx.enter_context(tc.tile_pool(name="y3p", bufs=8))
        o3p = ctx.enter_context(tc.tile_pool(name="o3p", bufs=3))
        dsm = ctx.enter_context(tc.tile_pool(name="dsm", bufs=4))
        dm3 = ctx.enter_context(tc.tile_pool(name="dm3", bufs=3))
        pp = ctx.enter_context(tc.tile_pool(name="pp", bufs=3, space="PSUM"))
        p3p = ctx.enter_context(tc.tile_pool(name="p3p", bufs=5, space="PSUM"))

        # ---- params in ----
        w1hs = wp.tile([128, 4, WID], BF16)
        nc.sync.dma_start(w1hs[:], w1h[:].rearrange("(k c) o -> c k o", c=128))
        w1ls = wp.tile([128, 4, WID], BF16)
        nc.sync.dma_start(w1ls[:], w1l[:].rearrange("(k c) o -> c k o", c=128))
        w2hs = wp.tile([128, 9, WID], BF16)
        nc.sync.dma_start(w2hs[:], w2h[:].rearrange("t c o -> c t o"))
        w2ls = wp.tile([128, 9, WID], BF16)
        nc.sync.dma_start(w2ls[:], w2l[:].rearrange("t c o -> c t o"))
        w3hs = wp.tile([128, CIN], BF16)
        nc.sync.dma_start(w3hs[:], w3h[:])
        w3ls = wp.tile([128, CIN], BF16)
        nc.sync.dma_start(w3ls[:], w3l[:])
        identsb = wp.tile([128, 128], BF16)
        nc.sync.dma_start(identsb[:], ident[:])
        bn1s = wp.tile([128, 1], F32); nc.sync.dma_start(bn1s[:], inv1[:])
        bn1b = wp.tile([128, 1], F32); nc.sync.dma_start(bn1b[:], bet1[:])
        bn2s = wp.tile([128, 1], F32); nc.sync.dma_start(bn2s[:], inv2[:])
        bn2b = wp.tile([128, 1], F32); nc.sync.dma_start(bn2b[:], bet2[:])

        # ---- activations / residual state ----
        a1pad = big.tile([128, N_IMG, PADH, PADW], BF16)
        nc.gpsimd.memset(a1pad[:].rearrange("p n h w -> p (n h w)").bitcast(I32), 0)
        a2 = big.tile([128, PIX], BF16)

        # ================= emit functions =================
        taps = [(dy, dx) for dy in range(3) for dx in range(3)]

        def emit_quant12(ygrp, dst_aps):
            """fp16 ygrp [128,1568] -> block-quantized bf16 into dst_aps."""
            rmax = dsm.tile([128, 49], F16, tag="rmax")
            nc.vector.tensor_reduce(rmax[:], ygrp[:].rearrange("p (b j) -> p b j", b=49, j=32),
                                    axis=mybir.AxisListType.X, op=AL.max,
                                    apply_transpose=True)
            nc.vector.tensor_scalar_max(rmax[:], rmax[:], RMIN)
            nc.vector.tensor_scalar(rmax.bitcast(I16), rmax.bitcast(I16),
                                    0x7C00, None, op0=AL.bitwise_and)
            nc.vector.tensor_scalar_mul(rmax[:], rmax[:], 0.015625)
            dcm = dsm.tile([128, GRP], F16, tag="dcm")
            nc.vector.transpose(dcm[:], rmax[:].unsqueeze(2).broadcast_to([128, 49, 32]))
            for im in range(2):
                nc.vector._custom_dve(
                    BFP_QUANT_ANT,
                    out=dst_aps[im],
                    in0=ygrp[:, im*HW:(im+1)*HW],
                    in1=dcm[:, im*HW:(im+1)*HW],
                    s0=MAGIC, s1=MAGIC + 127.0,
                )

        def emit_l1(g):
            ygrp = ygp.tile([128, GRP], F16, tag="ygrp")
            for si in range(4):
                n = 2 * g + si // 2
                q0 = NT392 * (si % 2)
                xk = []
                for k in range(4):
                    xt = xs.tile([128, NT392], BF16, tag="xk")
                    eng = nc.sync if (k % 2 == 0) else nc.scalar
                    eng.dma_start(xt[:], xh[n, 128*k:128*(k+1), q0:q0+NT392])
                    xk.append(xt)
                pst = pp.tile([128, CIN], F32, tag="cp")
                ps = pst[:, :NT392]
                nmm = 8 if W1_LO else 4
                i = 0
                for k in range(4):
                    for wt in ((w1hs, w1ls) if W1_LO else (w1hs,)):
                        nc.tensor.matmul(ps[:], wt[:, k, :], xk[k][:, :],
                                         start=(i == 0), stop=(i == nmm - 1))
                        i += 1
                nc.scalar.activation(ygrp[:, si*NT392:(si+1)*NT392], ps[:], AFT.Relu,
                                     bias=bn1b[:], scale=bn1s[:])
            n0 = 2 * g
            emit_quant12(ygrp, [a1pad[:, n0, 1:29, 1:29], a1pad[:, n0+1, 1:29, 1:29]])

        def emit_l2(g):
            ygrp = ygp.tile([128, GRP], F16, tag="y2grp")
            w2list = (w2hs, w2ls) if W2_LO else (w2hs,)
            nmm = 9 * len(w2list)
            for si in range(4):
                n = 2 * g + si // 2
                h0 = 14 * (si % 2)
                pst = pp.tile([128, CIN], F32, tag="cp")
                ps = pst[:, :NT392]
                i = 0
                for t, (dy, dx) in enumerate(taps):
                    for wt in w2list:
                        rhs = a1pad[:, n, h0+dy:h0+dy+14, dx:dx+28]
                        nc.tensor.matmul(ps[:], wt[:, t, :], rhs,
                                         start=(i == 0), stop=(i == nmm - 1))
                        i += 1
                nc.scalar.activation(ygrp[:, si*NT392:(si+1)*NT392], ps[:], AFT.Relu,
                                     bias=bn2b[:], scale=bn2s[:])
            n0 = 2 * g
            emit_quant12(ygrp, [a2[:, n0*HW:(n0+1)*HW], a2[:, (n0+1)*HW:(n0+2)*HW]])

        def emit_l3(t0, gn):
            """gn (<=4) consecutive 128-pixel tiles starting at tile t0."""
            nf = gn * CIN
            xr = xt3.tile([128, 4 * CIN], BF16, tag="xr")
            nc.sync.dma_start(xr[:, :nf].rearrange("p (j c) -> p j c", j=gn, c=CIN),
                              xT[128*t0:128*t0 + 128*gn, :].rearrange("(j p) c -> p j c", p=128))
            rm16 = dm3.tile([128, 4 * 16], F16, tag="rm16")
            ys = []
            for j in range(gn):
                ps3 = p3p.tile([128, CIN], F32, tag="c3g")
                a2t = a2[:, 128*(t0+j):128*(t0+j+1)]
                if W3_LO:
                    nc.tensor.matmul(ps3[:], a2t, w3hs[:], start=True, stop=False)
                    nc.tensor.matmul(ps3[:], a2t, w3ls[:], start=False, stop=False)
                else:
                    nc.tensor.matmul(ps3[:], a2t, w3hs[:], start=True, stop=False)
                nc.tensor.matmul(ps3[:], identsb[:], xr[:, j*CIN:(j+1)*CIN],
                                 start=False, stop=True)
                y16 = y3p.tile([128, CIN], F16, tag="y16")
                nc.scalar.activation(y16[:], ps3[:], AFT.Relu)
                nc.vector.tensor_reduce(rm16[:, j*16:(j+1)*16],
                                        y16[:].rearrange("p (b k) -> p b k", b=16, k=32),
                                        axis=mybir.AxisListType.X, op=AL.max)
                ys.append(y16)
            # batched delta math: fp32 exponent-floor, then back to fp16
            dl32 = dm3.tile([128, 4 * 16], F32, tag="dl32")
            nc.vector.tensor_copy(dl32[:, :16*gn], rm16[:, :16*gn])
            nc.vector.tensor_scalar_max(dl32[:, :16*gn], dl32[:, :16*gn], RMIN)
            nc.vector.tensor_scalar(dl32.bitcast(I32)[:, :16*gn], dl32.bitcast(I32)[:, :16*gn],
                                    0x7F800000, None, op0=AL.bitwise_and)
            dl16 = dm3.tile([128, 4 * 16], F16, tag="dl16")
            nc.vector.tensor_scalar(dl16[:, :16*gn], dl32[:, :16*gn], 0.015625, None,
                                    op0=AL.mult)
            o3 = o3p.tile([128, 4 * CIN], F16, tag="o3")
            for j in range(gn):
                nc.vector._custom_dve(
                    BFP_QUANT_ANT,
                    out=o3[:, j*CIN:(j+1)*CIN].rearrange("p (b k) -> p b k", b=16, k=32),
                    in0=ys[j][:].rearrange("p (b k) -> p b k", b=16, k=32),
                    in1=dl16[:, j*16:(j+1)*16].unsqueeze(2).broadcast_to([128, 16, 32]),
                    s0=MAGIC, s1=MAGIC + 127.0,
                )
            nc.scalar.dma_start(outV[128*t0:128*t0 + 128*gn, :].rearrange("(j p) c -> p j c", p=128),
                                o3[:, :nf].rearrange("p (j c) -> p j c", j=gn, c=CIN))

        # ================= interleaved schedule =================
        l3g = [(4*i, min(4, 49 - 4*i)) for i in range((49 + 3) // 4)]  # 13 groups
        emit_l1(0)
        emit_l1(1)
        emit_l2(0)
        emit_l1(2)
        for t0, gn in l3g[:3]:      # tiles 0..11, needs quant2(0) only
            emit_l3(t0, gn)
        emit_l2(1)
        emit_l1(3)
        for t0, gn in l3g[3:6]:     # tiles 12..23, needs quant2(1)
            emit_l3(t0, gn)
        emit_l2(2)
        for t0, gn in l3g[6:9]:     # tiles 24..35, needs quant2(2)
            emit_l3(t0, gn)
        emit_l2(3)
        for t0, gn in l3g[9:]:      # tiles 36..48
            emit_l3(t0, gn)

    nc.finalize()
    return nc


# ---------------- host-side parameter prep ---------------------------------
def _w_quant_np(w, blk=32):
    O, I, kh, kw = w.shape
    wb = w.reshape(O, I // blk, blk, kh, kw)
    alpha = np.maximum(np.abs(wb).max(axis=2, keepdims=True) / np.float32(127.0),
                       np.float32(1e-24)).astype(np.float32)
    q = (np.round(wb / alpha) * alpha).astype(np.float32)
    return q.reshape(O, I, kh, kw)


def _bn_fold(g, b, m, v):
    inv = (g / np.sqrt(v + np.float32(1e-5))).astype(np.float32)
    beta = (b - m * inv).astype(np.float32)
    return inv, beta


def _hilo(w):
    hi = w.astype(ml_dtypes.bfloat16)
    lo = (w - hi.astype(np.float32)).astype(ml_dtypes.bfloat16)
    return hi, lo


_NC_CACHE = {}

def kernel(x, w1, w2, w3,
           bn1_g, bn1_b, bn1_m, bn1_v,
           bn2_g, bn2_b, bn2_m, bn2_v,
           bn3_g, bn3_b, bn3_m, bn3_v,
           _want_trace=False):
    x = np.asarray(x, np.float32)
    w1q = _w_quant_np(np.asarray(w1, np.float32))
    w2q = _w_quant_np(np.asarray(w2, np.float32))
    w3q = _w_quant_np(np.asarray(w3, np.float32))
    inv1, bet1 = _bn_fold(*[np.asarray(a, np.float32) for a in (bn1_g, bn1_b, bn1_m, bn1_v)])
    inv2, bet2 = _bn_fold(*[np.asarray(a, np.float32) for a in (bn2_g, bn2_b, bn2_m, bn2_v)])
    inv3, bet3 = _bn_fold(*[np.asarray(a, np.float32) for a in (bn3_g, bn3_b, bn3_m, bn3_v)])

    # bn3 beta folded into the residual input; conv1 bias corrected for it
    xb3 = (x + bet3[None, :, None, None]).astype(np.float32)
    K = (w1q[:, :, 0, 0].astype(np.float64) @ bet3.astype(np.float64))
    bet1c = (bet1.astype(np.float64) - inv1.astype(np.float64) * K).astype(np.float32)

    w1T = np.ascontiguousarray(w1q[:, :, 0, 0].T)                     # [512, 128]
    w2T = np.ascontiguousarray(w2q.transpose(2, 3, 1, 0).reshape(9, WID, WID))
    w3f = (w3q[:, :, 0, 0] * inv3[:, None]).astype(np.float32)
    w3T = np.ascontiguousarray(w3f.T)                                 # [128, 512]

    w1hi, w1lo = _hilo(w1T)
    w2hi, w2lo = _hilo(w2T)
    w3hi, w3lo = _hilo(w3T)

    xb3v = xb3.reshape(64, CIN, HW)
    xh_all = xb3v.astype(ml_dtypes.bfloat16)                          # ch-major bf16
    xT_all = np.ascontiguousarray(
        xb3v.transpose(0, 2, 1)).astype(ml_dtypes.bfloat16)           # [64, 784, 512]

    if "nc" not in _NC_CACHE:
        _NC_CACHE["nc"] = build_nc()
    nc = _NC_CACHE["nc"]

    shared = dict(
        w1h=w1hi, w1l=w1lo, w2h=w2hi, w2l=w2lo, w3h=w3hi, w3l=w3lo,
        ident=np.eye(128, dtype=ml_dtypes.bfloat16),
        inv1=inv1.reshape(WID, 1), bet1=bet1c.reshape(WID, 1),
        inv2=inv2.reshape(WID, 1), bet2=bet2.reshape(WID, 1),
    )
    in_maps = []
    for c in range(8):
        m = dict(shared)
        m["xh"] = np.ascontiguousarray(xh_all[8*c:8*(c+1)])
        m["xT"] = np.ascontiguousarray(xT_all[8*c:8*(c+1)].reshape(PIX, CIN))
        in_maps.append(m)

    res = run_bass_kernel_spmd(nc, in_maps, list(range(8)), trace=_want_trace)
    out = np.empty((64, CIN, H, W), np.float32)
    for c in range(8):
        oT = res.results[c]["outV"].astype(np.float32).reshape(N_IMG, HW, CIN)
        out[8*c:8*(c+1)] = oT.transpose(0, 2, 1).reshape(N_IMG, CIN, H, W)
    if _want_trace:
        return out, res
    return out
